# revision 17
# baseline (speedup 1.0000x reference)
"""Trainium2 Bass kernel for nn_Net_26912265076915 (submanifold sparse conv net).

Dense masked-conv strategy, channel-major [C, z, y, x] with zero-padded
shells. Each conv = 27 shifted-AP matmuls accumulated in PSUM; 0/1 activity
masks at every conv output enforce submanifold semantics exactly; max-pools
see -BIG at inactive sites (negbias folded into the previous conv's output).
Levels 0-3 z-sharded across 8 cores with uniform local plane coordinates
(per-core input windows come from the host); halo planes patched at level
entry via AllGather of boundary strips. Levels 4-6 replicated on all cores.
"""

import numpy as np
import ml_dtypes

import concourse.bass as bass
import concourse.bacc as bacc
import concourse.mybir as mybir
import concourse.tile as tile
from concourse.bass_utils import run_bass_kernel_spmd

SHAPE = (128, 128, 128)
NCORES = 8
CONV_CH = [(3, 64), (64, 64), (64, 96), (96, 96), (96, 128), (128, 128),
           (128, 160), (160, 160), (160, 192), (192, 192), (192, 224),
           (224, 224), (224, 256), (256, 256)]
LEVELS = 7
SIDE = [128 >> l for l in range(LEVELS)]
SHARDED = [l <= 3 for l in range(LEVELS)]
CONV_LEVEL = [l for l in range(LEVELS) for _ in range(2)]
BIG = 1.0e30
F32 = mybir.dt.float32
BF16 = mybir.dt.bfloat16
BF16NP = ml_dtypes.bfloat16


def _dims(l):
    s = SIDE[l]
    return s + 2, s + 2, s + 2


def _yx(l):
    _, Yp, Xp = _dims(l)
    return Yp * Xp


def _slab(l):
    return SIDE[l] // NCORES if SHARDED[l] else SIDE[l]


def _locp(l):
    return _slab(l) + 6 if SHARDED[l] else SIDE[l] + 2


def _kb(c):
    return (c + 127) // 128


# ---------------------------------------------------------------------------
# Host plan
# ---------------------------------------------------------------------------

def build_host_plan(coors_np):
    active = []
    coords = coors_np.astype(np.int64)
    for l in range(LEVELS):
        s = SIDE[l]
        act = np.zeros((s, s, s), dtype=bool)
        act[coords[:, 1], coords[:, 2], coords[:, 3]] = True
        active.append(act)
        if l < LEVELS - 1:
            c2 = coords.copy()
            c2[:, 1:] = c2[:, 1:] // 2
            s2 = s // 2
            key = ((c2[:, 0] * s2 + c2[:, 1]) * s2 + c2[:, 2]) * s2 + c2[:, 3]
            uniq = np.unique(key)
            b = uniq // (s2 * s2 * s2)
            r = uniq % (s2 * s2 * s2)
            z = r // (s2 * s2)
            r = r % (s2 * s2)
            coords = np.stack([b, z, r // s2, r % s2], 1)
    s = SIDE[LEVELS - 1]
    zz, yy, xx = np.nonzero(active[LEVELS - 1])
    lin = (zz * s + yy) * s + xx
    order = np.argsort(lin)
    final_sites = np.stack([zz[order], yy[order], xx[order]], 1)
    return active, final_sites


def _padded_mask(act):
    s = act.shape[0]
    m = np.zeros((s + 2, s + 2, s + 2), np.float32)
    m[1:-1, 1:-1, 1:-1] = act.astype(np.float32)
    return m


def _core_window(l, core, arr):
    """Window [locp, Yp, Xp] of full padded [Zp, Yp, Xp] array for a core.
    Local plane p <-> global padded plane z0 + p - 2."""
    Zp, Yp, Xp = _dims(l)
    if not SHARDED[l]:
        return arr
    locp = _locp(l)
    z0 = core * _slab(l)
    out = np.zeros((locp, Yp, Xp), arr.dtype)
    for p in range(locp):
        gz = z0 + p - 2
        if 0 <= gz < Zp:
            out[p] = arr[gz]
    return out


def pack_weights(ws):
    packed = []
    for wi, w in enumerate(ws):
        if wi == 0:
            w = np.asarray(w, np.float32)  # [27, 3, 64]
            buf = np.zeros((128, 128), np.float32)
            for j in range(27):
                buf[j * 4:j * 4 + 3, 0:w.shape[2]] = w[j]
            packed.append(buf.astype(BF16NP))
            continue
        w = np.asarray(w, np.float32)
        cin, cout = w.shape[1], w.shape[2]
        nkb, nmb = _kb(cin), _kb(cout)
        buf = np.zeros((nkb * 128, 27 * nmb * 128), np.float32)
        for kb in range(nkb):
            klo, khi = kb * 128, min(cin, kb * 128 + 128)
            for mb in range(nmb):
                mlo, mhi = mb * 128, min(cout, mb * 128 + 128)
                for k in range(27):
                    col = (k * nmb + mb) * 128
                    buf[kb * 128:kb * 128 + (khi - klo),
                        col:col + (mhi - mlo)] = w[k, klo:khi, mlo:mhi]
        packed.append(buf.astype(BF16NP))
    return packed


# ---------------------------------------------------------------------------
# Device program (static; all data arrives via inputs)
# ---------------------------------------------------------------------------

_PROG_CACHE = {}


def build_program():
    if "p" in _PROG_CACHE:
        return _PROG_CACHE["p"]
    nc = bacc.Bacc("TRN2", target_bir_lowering=False, debug=False,
                   num_devices=NCORES)

    g0 = nc.dram_tensor("grid0", [4, _locp(0) * _yx(0)], BF16,
                        kind="ExternalInput")
    w_in = [nc.dram_tensor(
        f"w{i}",
        [128, _kb(co) * 128] if i == 0 else [_kb(ci) * 128,
                                             27 * _kb(co) * 128],
        BF16, kind="ExternalInput")
        for i, (ci, co) in enumerate(CONV_CH)]
    mask_in = [nc.dram_tensor(f"mask{l}", [1, _locp(l) * _yx(l)], BF16,
                              kind="ExternalInput") for l in range(LEVELS)]
    nbias_in = [nc.dram_tensor(f"nbias{l}", [1, _locp(l) * _yx(l)], BF16,
                               kind="ExternalInput")
                for l in range(LEVELS - 1)]
    sel_top_t = nc.dram_tensor("sel_top", [128, 8], F32, kind="ExternalInput")
    sel_bot_t = nc.dram_tensor("sel_bot", [128, 8], F32, kind="ExternalInput")
    out6 = nc.dram_tensor("out6", [256, _locp(6) * _yx(6)], F32,
                          kind="ExternalOutput")

    with tile.TileContext(nc) as tc:
        with (tc.tile_pool(name="dram", bufs=1, space="DRAM") as dram,
              tc.tile_pool(name="slots", bufs=1) as slots_pool,
              tc.tile_pool(name="sb", bufs=3) as sb,
              tc.tile_pool(name="sbp", bufs=2) as sbp,
              tc.tile_pool(name="wpool", bufs=1) as wpool,
              tc.tile_pool(name="ps", bufs=4, space="PSUM") as ps):

            cbuf = [dram.tile([_kb(co) * 128, _locp(CONV_LEVEL[i]) *
                               _yx(CONV_LEVEL[i])], BF16, tag=f"cbuf{i}",
                              name=f"cbuf{i}")
                    for i, (ci, co) in enumerate(CONV_CH)]
            lin = [None] + [dram.tile([_kb(CONV_CH[2 * l][0]) * 128,
                                       _locp(l) * _yx(l)], BF16,
                                      tag=f"lin{l}", name=f"lin{l}")
                            for l in range(1, LEVELS)]

            sel_top = sb.tile([128, 8], F32, tag="selt")
            sel_bot = sb.tile([128, 8], F32, tag="selb")
            nc.sync.dma_start(sel_top[:, :], sel_top_t.ap())
            nc.sync.dma_start(sel_bot[:, :], sel_bot_t.ap())

            zt = wpool.tile([128, 2048], BF16, tag="zt")
            nc.vector.memset(zt[:, :], 0.0)
            ones = wpool.tile([1, 128], BF16, tag="ones")
            nc.vector.memset(ones[:, :], 1.0)

            def bcast_mask(mrow, cw):
                mps = ps.tile([128, 512], F32, tag="mps")
                nc.tensor.matmul(mps[0:128, 0:cw], ones[0:1, 0:128],
                                 mrow[0:1, 0:cw], start=True, stop=True)
                mb = sb.tile([128, 512], BF16, tag="mbuf")
                nc.vector.tensor_scalar_mul(mb[:, 0:cw], mps[:, 0:cw], 1.0)
                return mb

            def memset_shell_planes(buf, nkb, locp, YX):
                for kb in range(nkb):
                    for p in (0, locp - 1):
                        off = 0
                        while off < YX:
                            w = min(2048, YX - off)
                            nc.sync.dma_start(
                                buf[kb * 128:(kb + 1) * 128,
                                    p * YX + off:p * YX + off + w],
                                zt[:, 0:w])
                            off += w

            def conv(ci, src, plo, phi, is_final=False):
                l = CONV_LEVEL[ci]
                cin, cout = CONV_CH[ci]
                nkb, nmb = _kb(cin), _kb(cout)
                Zp, Yp, Xp = _dims(l)
                YX = _yx(l)
                slot_n = YX + 2 * (Xp + 1)
                kp = 4 if ci == 0 else 128
                apply_nb = (ci % 2 == 1) and l < LEVELS - 1

                stacked = (ci == 0)
                if stacked:
                    wt = wpool.tile([128, 128], BF16, tag="wts")
                    nc.sync.dma_start(wt[:, :], w_in[0].ap())
                else:
                    wt = wpool.tile([128, nkb * 27 * nmb * 128], BF16,
                                    tag="wts")
                    nc.sync.dma_start(
                        wt[:, :].rearrange("p (a b) -> p a b", a=nkb),
                        w_in[ci].ap().rearrange("(a p) b -> p a b", p=128))

                offs = [(dz, dy * Xp + dx)
                        for dz in (-1, 0, 1) for dy in (-1, 0, 1)
                        for dx in (-1, 0, 1)]
                nch = (YX + 511) // 512
                dst = cbuf[ci]
                slot_tiles = {}
                for p in range(plo, phi):
                    if stacked:
                        sst = slots_pool.tile([128, YX], BF16,
                                              tag=f"slot{p % 3}")
                        for j, (dz, dfree) in enumerate(offs):
                            st0 = (p + dz) * YX + dfree
                            nc.sync.dma_start(
                                sst[j * 4:j * 4 + 4, 0:YX],
                                src[0:4, st0:st0 + YX])
                        for c in range(nch):
                            c0 = c * 512
                            cw = min(512, YX - c0)
                            mrow = sb.tile([1, 512], BF16, tag="mrow")
                            nc.sync.dma_start(
                                mrow[0:1, 0:cw],
                                mask_in[l].ap()[0:1,
                                                p * YX + c0:p * YX + c0 + cw])
                            mbt = bcast_mask(mrow, cw)
                            acc = ps.tile([128, 512], F32, tag="acc")
                            nc.tensor.matmul(
                                acc[0:128, 0:cw], wt[0:108, 0:128],
                                sst[0:108, c0:c0 + cw],
                                start=True, stop=True)
                            ot = sb.tile([128, 512], BF16, tag="evac")
                            nc.vector.tensor_tensor(
                                ot[:, 0:cw], acc[:, 0:cw], mbt[:, 0:cw],
                                op=mybir.AluOpType.mult)
                            nc.sync.dma_start(
                                dst[0:128, p * YX + c0:p * YX + c0 + cw],
                                ot[:, 0:cw])
                        continue
                    for q in (p - 1, p, p + 1):
                        if q not in slot_tiles:
                            st = slots_pool.tile([128, nkb * slot_n], BF16,
                                                 tag=f"slot{q % 3}")
                            total = _locp(l) * YX
                            for kb in range(nkb):
                                base = q * YX - (Xp + 1)
                                lo = max(base, 0)
                                hi = min(base + slot_n, total)
                                if lo > base or hi < base + slot_n:
                                    off = 0
                                    while off < slot_n:
                                        w = min(2048, slot_n - off)
                                        nc.vector.tensor_copy(
                                            st[0:kp, kb * slot_n + off:
                                               kb * slot_n + off + w],
                                            zt[0:kp, 0:w])
                                        off += w
                                nc.sync.dma_start(
                                    st[0:kp, kb * slot_n + (lo - base):
                                       kb * slot_n + (hi - base)],
                                    src[kb * 128:kb * 128 + kp, lo:hi])
                            slot_tiles[q] = st
                    for q in list(slot_tiles):
                        if q < p - 1:
                            del slot_tiles[q]
                    for c in range(nch):
                        c0 = c * 512
                        cw = min(512, YX - c0)
                        mrow = sb.tile([1, 512], BF16, tag="mrow")
                        nc.sync.dma_start(
                            mrow[0:1, 0:cw],
                            mask_in[l].ap()[0:1, p * YX + c0:p * YX + c0 + cw])
                        mbt = bcast_mask(mrow, cw)
                        for mb in range(nmb):
                            acc = ps.tile([128, 512], F32, tag="acc")
                            n_mm = nkb * 27
                            i_mm = 0
                            for kb in range(nkb):
                                for k27, (dz, dfree) in enumerate(offs):
                                    woff = kb * 27 * nmb * 128 \
                                        + (k27 * nmb + mb) * 128
                                    rb = kb * slot_n + (Xp + 1) + c0 + dfree
                                    nc.tensor.matmul(
                                        acc[0:128, 0:cw],
                                        wt[0:kp, woff:woff + 128],
                                        slot_tiles[p + dz][0:kp, rb:rb + cw],
                                        start=(i_mm == 0),
                                        stop=(i_mm == n_mm - 1))
                                    i_mm += 1
                            ot = sb.tile([128, 512],
                                         F32 if is_final else BF16,
                                         tag="evacf" if is_final else "evac")
                            nc.vector.tensor_tensor(
                                ot[:, 0:cw], acc[:, 0:cw], mbt[:, 0:cw],
                                op=mybir.AluOpType.mult)
                            if apply_nb:
                                nbr = sb.tile([128, 512], BF16, tag="nbuf")
                                nc.vector.tensor_scalar(
                                    nbr[:, 0:cw], mbt[:, 0:cw], BIG, -BIG,
                                    op0=mybir.AluOpType.mult,
                                    op1=mybir.AluOpType.add)
                                nc.vector.tensor_tensor(
                                    ot[:, 0:cw], ot[:, 0:cw], nbr[:, 0:cw],
                                    op=mybir.AluOpType.add)
                            tgt = out6.ap() if is_final else dst
                            nc.sync.dma_start(
                                tgt[mb * 128:(mb + 1) * 128,
                                    p * YX + c0:p * YX + c0 + cw],
                                ot[:, 0:cw])

            def pool(l, Plo, Phi, stage=None, write_lin=True):
                """Max-pool level l -> l+1 over next-level local planes
                [Plo, Phi). stage: (stage_tile, plane->slot map)."""
                nl = l + 1
                src_ci = 2 * l + 1
                cout = CONV_CH[src_ci][1]
                nkb = _kb(cout)
                Zp, Yp, Xp = _dims(l)
                YX = _yx(l)
                Yp2, Xp2 = _dims(nl)[1:]
                YX2 = _yx(nl)
                S2 = SIDE[nl]
                src = cbuf[src_ci]
                for P in range(Plo, Phi):
                    pin = (2 * P - 3) if SHARDED[l] else (2 * P - 1)
                    for kb in range(nkb):
                        wa = slots_pool.tile([128, YX], BF16, tag="slot0")
                        wb = slots_pool.tile([128, YX], BF16, tag="slot1")
                        nc.sync.dma_start(
                            wa[:, :], src[kb * 128:(kb + 1) * 128,
                                          pin * YX:(pin + 1) * YX])
                        nc.sync.dma_start(
                            wb[:, :], src[kb * 128:(kb + 1) * 128,
                                          (pin + 1) * YX:(pin + 2) * YX])
                        o = sbp.tile([128, YX2], BF16, tag="pout")
                        nc.vector.memset(o[:, :], 0.0)
                        ov = o[0:128, 0:YX2].rearrange(
                            "p (y x) -> p y x", y=Yp2)[0:128, 1:S2 + 1,
                                                       1:S2 + 1]
                        first = True
                        for oz in (0, 1):
                            wz = (wa, wb)[oz]
                            wv = wz[0:128, 0:YX].rearrange(
                                "p (y x) -> p y x", y=Yp)
                            for oy in (0, 1):
                                for ox in (0, 1):
                                    s_ap = wv[0:128,
                                              1 + oy:1 + oy + 2 * S2:2,
                                              1 + ox:1 + ox + 2 * S2:2]
                                    if first:
                                        nc.vector.tensor_tensor(
                                            ov, s_ap, s_ap,
                                            op=mybir.AluOpType.max)
                                        first = False
                                    else:
                                        nc.vector.tensor_tensor(
                                            ov, ov, s_ap,
                                            op=mybir.AluOpType.max)
                        for c2 in range(0, YX2, 512):
                            w2 = min(512, YX2 - c2)
                            mrow2 = sb.tile([1, 512], BF16, tag="mrow2")
                            nc.sync.dma_start(
                                mrow2[0:1, 0:w2],
                                mask_in[nl].ap()[0:1, P * YX2 + c2:
                                                 P * YX2 + c2 + w2])
                            mb2 = bcast_mask(mrow2, w2)
                            nc.vector.tensor_tensor(
                                o[:, c2:c2 + w2], o[:, c2:c2 + w2],
                                mb2[:, 0:w2],
                                op=mybir.AluOpType.mult)
                        om = o
                        if write_lin:
                            nc.sync.dma_start(
                                lin[nl][kb * 128:(kb + 1) * 128,
                                        P * YX2:(P + 1) * YX2],
                                om[:, :])
                        if stage is not None:
                            st_tile, slotmap = stage
                            if P in slotmap:
                                for s in slotmap[P]:
                                    nc.sync.dma_start(
                                        st_tile[:, (s * nkb + kb) * YX2:
                                                (s * nkb + kb + 1) * YX2],
                                        om[:, :])

            def halo_exchange(nl, stage_tile, nkb):
                """AllGather stage strips and patch halo planes of lin[nl].
                Neighbor selection is static: weighted sum over all 8 ranks
                with per-core 0/1 select weights (avoids dynamic APs)."""
                YX2 = _yx(nl)
                slab2 = _slab(nl)
                W = 4 * nkb * YX2
                ago = dram.tile([8 * 128, W], BF16, tag=f"ago{nl}",
                                name=f"ago{nl}")
                nc.gpsimd.collective_compute(
                    "AllGather", mybir.AluOpType.bypass,
                    replica_groups=[list(range(NCORES))],
                    ins=[stage_tile[:, :].opt()],
                    outs=[ago[:, :].opt()])
                # top halo: rank (c-1)'s bottom strips (slots 2,3) -> planes
                # 1,2; bottom halo: rank (c+1)'s top strips (slots 0,1) ->
                # planes slab2+3, slab2+4
                for side, selw in ((0, sel_top), (1, sel_bot)):
                    for s in (0, 1):
                        slot = (2 + s) if side == 0 else s
                        dstp = (1 + s) if side == 0 else (slab2 + 3 + s)
                        for kb in range(nkb):
                            acc = sbp.tile([128, YX2], BF16, tag="pout")
                            for r in range(NCORES):
                                hb = sbp.tile([128, YX2], BF16, tag="hbx")
                                nc.sync.dma_start(
                                    hb[:, :],
                                    ago[r * 128:(r + 1) * 128,
                                        (slot * nkb + kb) * YX2:
                                        (slot * nkb + kb + 1) * YX2])
                                if r == 0:
                                    nc.vector.tensor_scalar_mul(
                                        acc[:, :], hb[:, :],
                                        selw[0:128, r:r + 1])
                                else:
                                    tmp = sbp.tile([128, YX2], BF16,
                                                   tag="hby")
                                    nc.vector.tensor_scalar_mul(
                                        tmp[:, :], hb[:, :],
                                        selw[0:128, r:r + 1])
                                    nc.vector.tensor_tensor(
                                        acc[:, :], acc[:, :], tmp[:, :],
                                        op=mybir.AluOpType.add)
                            nc.sync.dma_start(
                                lin[nl][kb * 128:(kb + 1) * 128,
                                        dstp * YX2:(dstp + 1) * YX2],
                                acc[:, :])

            # ================= network =================
            # L0
            s0 = _slab(0)
            conv(0, g0.ap(), 2, s0 + 4)
            conv(1, cbuf[0], 3, s0 + 3)
            for l in (0, 1, 2):
                nl = l + 1
                slab2 = _slab(nl)
                nkb2 = _kb(CONV_CH[2 * nl][0])
                stage = dram.tile([128, 4 * nkb2 * _yx(nl)], BF16,
                                  tag=f"stage{nl}")
                sm = {}
                for plane, slot in ((3, 0), (4, 1), (slab2 + 1, 2),
                                    (slab2 + 2, 3)):
                    sm.setdefault(plane, []).append(slot)
                pool(l, 3, slab2 + 3, stage=(stage, sm))
                halo_exchange(nl, stage, nkb2)
                sl2 = _slab(nl)
                conv(2 * nl, lin[nl], 2, sl2 + 4)
                conv(2 * nl + 1, cbuf[2 * nl], 3, sl2 + 3)
            # L3 -> L4 (full gather of single planes)
            nkb4 = _kb(CONV_CH[8][0])
            YX4 = _yx(4)
            stage4 = dram.tile([128, nkb4 * YX4], BF16, tag="stage4")
            sm4 = {3: [0]}
            pool(3, 3, 4, stage=(stage4, sm4), write_lin=False)
            ago4 = dram.tile([8 * 128, nkb4 * YX4], BF16, tag="ago4")
            nc.gpsimd.collective_compute(
                "AllGather", mybir.AluOpType.bypass,
                replica_groups=[list(range(NCORES))],
                ins=[stage4[:, :].opt()],
                outs=[ago4[:, :].opt()])
            memset_shell_planes(lin[4], nkb4, _locp(4), YX4)
            for r in range(NCORES):
                for kb in range(nkb4):
                    hb = sbp.tile([128, YX4], BF16, tag="pout")
                    nc.sync.dma_start(
                        hb[:, :], ago4[r * 128:(r + 1) * 128,
                                       kb * YX4:(kb + 1) * YX4])
                    nc.sync.dma_start(
                        lin[4][kb * 128:(kb + 1) * 128,
                               (r + 1) * YX4:(r + 2) * YX4],
                        hb[:, :])
            # L4-L6 replicated
            for l in (4, 5, 6):
                Zp = _dims(l)[0]
                ca, cb = 2 * l, 2 * l + 1
                memset_shell_planes(cbuf[ca], _kb(CONV_CH[ca][1]), _locp(l),
                                    _yx(l))
                conv(ca, lin[l], 1, Zp - 1)
                conv(cb, cbuf[ca], 1, Zp - 1, is_final=(l == 6))
                if l < 6:
                    Zp2 = _dims(l + 1)[0]
                    memset_shell_planes(lin[l + 1], _kb(CONV_CH[2 * l + 2][0]),
                                        _locp(l + 1), _yx(l + 1))
                    pool(l, 1, Zp2 - 1)

    nc.compile()
    _PROG_CACHE["p"] = nc
    return nc


# ---------------------------------------------------------------------------
# Entry point
# ---------------------------------------------------------------------------

def kernel(**inputs):
    feats = np.asarray(inputs["features"], np.float32)
    coors = np.asarray(inputs["coors"], np.int32)
    ws = [np.asarray(inputs[f"w{i}"], np.float32) for i in range(len(CONV_CH))]

    active, final_sites = build_host_plan(coors)
    wpk = pack_weights(ws)

    # densified, padded level-0 input [4, Zp, Yp, Xp]
    Zp, Yp, Xp = _dims(0)
    dense0 = np.zeros((4, Zp, Yp, Xp), np.float32)
    dense0[0:3, coors[:, 1] + 1, coors[:, 2] + 1, coors[:, 3] + 1] = feats.T

    masks = [_padded_mask(active[l]) for l in range(LEVELS)]
    nbias = [(m - 1.0) * BIG for m in masks]

    nc = build_program()

    in_maps = []
    for core in range(NCORES):
        im = {}
        g0w = np.stack([_core_window(0, core, dense0[c]) for c in range(4)])
        im["grid0"] = g0w.reshape(4, -1).astype(BF16NP)
        for i in range(len(CONV_CH)):
            im[f"w{i}"] = wpk[i]
        for l in range(LEVELS):
            im[f"mask{l}"] = _core_window(l, core, masks[l]) \
                .reshape(1, -1).astype(BF16NP)
            if l < LEVELS - 1:
                im[f"nbias{l}"] = _core_window(l, core, nbias[l]) \
                    .reshape(1, -1).astype(BF16NP)
        st = np.zeros((128, 8), np.float32)
        if core > 0:
            st[:, core - 1] = 1.0
        sb_ = np.zeros((128, 8), np.float32)
        if core < NCORES - 1:
            sb_[:, core + 1] = 1.0
        im["sel_top"] = st
        im["sel_bot"] = sb_
        in_maps.append(im)

    import time as _time
    _t0 = _time.time()
    res = run_bass_kernel_spmd(nc, in_maps, core_ids=list(range(NCORES)))
    global LAST_EXEC_NS, LAST_RUN_WALL_S
    LAST_RUN_WALL_S = _time.time() - _t0
    LAST_EXEC_NS = res.exec_time_ns
    out = res.results[0]["out6"]  # [256, locp6*yx6]

    Yp6, Xp6 = _dims(6)[1:]
    cols = ((final_sites[:, 0] + 1) * Yp6 + final_sites[:, 1] + 1) * Xp6 \
        + final_sites[:, 2] + 1
    return np.ascontiguousarray(out[:, cols].T).astype(np.float32)


# revision 19
# speedup vs baseline: 1.1707x; 1.1707x over previous
"""Trainium2 Bass kernel for nn_Net_26912265076915 (submanifold sparse conv net).

Dense masked-conv strategy, channel-major [C, z, y, x] with zero-padded
shells. Each conv = 27 shifted-AP matmuls accumulated in PSUM; 0/1 activity
masks at every conv output enforce submanifold semantics exactly; max-pools
see -BIG at inactive sites (negbias folded into the previous conv's output).
Levels 0-3 z-sharded across 8 cores with uniform local plane coordinates
(per-core input windows come from the host); halo planes patched at level
entry via AllGather of boundary strips. Levels 4-6 replicated on all cores.
"""

import numpy as np
import ml_dtypes

import concourse.bass as bass
import concourse.bacc as bacc
import concourse.mybir as mybir
import concourse.tile as tile
from concourse.bass_utils import run_bass_kernel_spmd

SHAPE = (128, 128, 128)
NCORES = 8
CONV_CH = [(3, 64), (64, 64), (64, 96), (96, 96), (96, 128), (128, 128),
           (128, 160), (160, 160), (160, 192), (192, 192), (192, 224),
           (224, 224), (224, 256), (256, 256)]
LEVELS = 7
SIDE = [128 >> l for l in range(LEVELS)]
SHARDED = [l <= 3 for l in range(LEVELS)]
CONV_LEVEL = [l for l in range(LEVELS) for _ in range(2)]
BIG = 1.0e30
F32 = mybir.dt.float32
BF16 = mybir.dt.bfloat16
BF16NP = ml_dtypes.bfloat16


def _dims(l):
    s = SIDE[l]
    return s + 2, s + 2, s + 2


def _yx(l):
    _, Yp, Xp = _dims(l)
    return Yp * Xp


def _slab(l):
    return SIDE[l] // NCORES if SHARDED[l] else SIDE[l]


def _locp(l):
    return _slab(l) + 6 if SHARDED[l] else SIDE[l] + 2


def _kb(c):
    return (c + 127) // 128


# ---------------------------------------------------------------------------
# Host plan
# ---------------------------------------------------------------------------

def build_host_plan(coors_np):
    active = []
    coords = coors_np.astype(np.int64)
    for l in range(LEVELS):
        s = SIDE[l]
        act = np.zeros((s, s, s), dtype=bool)
        act[coords[:, 1], coords[:, 2], coords[:, 3]] = True
        active.append(act)
        if l < LEVELS - 1:
            c2 = coords.copy()
            c2[:, 1:] = c2[:, 1:] // 2
            s2 = s // 2
            key = ((c2[:, 0] * s2 + c2[:, 1]) * s2 + c2[:, 2]) * s2 + c2[:, 3]
            uniq = np.unique(key)
            b = uniq // (s2 * s2 * s2)
            r = uniq % (s2 * s2 * s2)
            z = r // (s2 * s2)
            r = r % (s2 * s2)
            coords = np.stack([b, z, r // s2, r % s2], 1)
    s = SIDE[LEVELS - 1]
    zz, yy, xx = np.nonzero(active[LEVELS - 1])
    lin = (zz * s + yy) * s + xx
    order = np.argsort(lin)
    final_sites = np.stack([zz[order], yy[order], xx[order]], 1)
    return active, final_sites


def _padded_mask(act):
    s = act.shape[0]
    m = np.zeros((s + 2, s + 2, s + 2), np.float32)
    m[1:-1, 1:-1, 1:-1] = act.astype(np.float32)
    return m


def _core_window(l, core, arr):
    """Window [locp, Yp, Xp] of full padded [Zp, Yp, Xp] array for a core.
    Local plane p <-> global padded plane z0 + p - 2."""
    Zp, Yp, Xp = _dims(l)
    if not SHARDED[l]:
        return arr
    locp = _locp(l)
    z0 = core * _slab(l)
    out = np.zeros((locp, Yp, Xp), arr.dtype)
    for p in range(locp):
        gz = z0 + p - 2
        if 0 <= gz < Zp:
            out[p] = arr[gz]
    return out


def pack_weights(ws):
    packed = []
    for wi, w in enumerate(ws):
        if wi == 0:
            w = np.asarray(w, np.float32)  # [27, 3, 64]
            buf = np.zeros((128, 128), np.float32)
            for j in range(27):
                buf[j * 4:j * 4 + 3, 0:w.shape[2]] = w[j]
            packed.append(buf.astype(BF16NP))
            continue
        w = np.asarray(w, np.float32)
        cin, cout = w.shape[1], w.shape[2]
        nkb, nmb = _kb(cin), _kb(cout)
        buf = np.zeros((nkb * 128, 27 * nmb * 128), np.float32)
        for kb in range(nkb):
            klo, khi = kb * 128, min(cin, kb * 128 + 128)
            for mb in range(nmb):
                mlo, mhi = mb * 128, min(cout, mb * 128 + 128)
                for k in range(27):
                    col = (k * nmb + mb) * 128
                    buf[kb * 128:kb * 128 + (khi - klo),
                        col:col + (mhi - mlo)] = w[k, klo:khi, mlo:mhi]
        if cin == 64:
            buf[64:128] = buf[0:64]
        packed.append(buf.astype(BF16NP))
    return packed


# ---------------------------------------------------------------------------
# Device program (static; all data arrives via inputs)
# ---------------------------------------------------------------------------

_PROG_CACHE = {}


def build_program():
    if "p" in _PROG_CACHE:
        return _PROG_CACHE["p"]
    nc = bacc.Bacc("TRN2", target_bir_lowering=False, debug=False,
                   num_devices=NCORES)

    g0 = nc.dram_tensor("grid0", [4, _locp(0) * _yx(0)], BF16,
                        kind="ExternalInput")
    w_in = [nc.dram_tensor(
        f"w{i}",
        [128, _kb(co) * 128] if i == 0 else [_kb(ci) * 128,
                                             27 * _kb(co) * 128],
        BF16, kind="ExternalInput")
        for i, (ci, co) in enumerate(CONV_CH)]
    mask_in = [nc.dram_tensor(f"mask{l}", [1, _locp(l) * _yx(l)], BF16,
                              kind="ExternalInput") for l in range(LEVELS)]
    nbias_in = [nc.dram_tensor(f"nbias{l}", [1, _locp(l) * _yx(l)], BF16,
                               kind="ExternalInput")
                for l in range(LEVELS - 1)]
    sel_top_t = nc.dram_tensor("sel_top", [128, 8], F32, kind="ExternalInput")
    sel_bot_t = nc.dram_tensor("sel_bot", [128, 8], F32, kind="ExternalInput")
    out6 = nc.dram_tensor("out6", [256, _locp(6) * _yx(6)], F32,
                          kind="ExternalOutput")

    with tile.TileContext(nc) as tc:
        with (tc.tile_pool(name="dram", bufs=1, space="DRAM") as dram,
              tc.tile_pool(name="slots", bufs=1) as slots_pool,
              tc.tile_pool(name="sb", bufs=3) as sb,
              tc.tile_pool(name="sbp", bufs=2) as sbp,
              tc.tile_pool(name="wpool", bufs=1) as wpool,
              tc.tile_pool(name="ps", bufs=2, space="PSUM") as ps):

            cbuf = [dram.tile([_kb(co) * 128, _locp(CONV_LEVEL[i]) *
                               _yx(CONV_LEVEL[i])], BF16, tag=f"cbuf{i}",
                              name=f"cbuf{i}")
                    for i, (ci, co) in enumerate(CONV_CH)]
            lin = [None] + [dram.tile([_kb(CONV_CH[2 * l][0]) * 128,
                                       _locp(l) * _yx(l)], BF16,
                                      tag=f"lin{l}", name=f"lin{l}")
                            for l in range(1, LEVELS)]

            sel_top = sb.tile([128, 8], F32, tag="selt")
            sel_bot = sb.tile([128, 8], F32, tag="selb")
            nc.sync.dma_start(sel_top[:, :], sel_top_t.ap())
            nc.sync.dma_start(sel_bot[:, :], sel_bot_t.ap())

            zt = wpool.tile([128, 2048], BF16, tag="zt")
            nc.vector.memset(zt[:, :], 0.0)
            ones = wpool.tile([1, 128], BF16, tag="ones")
            nc.vector.memset(ones[:, :], 1.0)

            def bcast_mask(mrow, cw, tp=None):
                mps = ps.tile([128, 512], F32, tag="mps")
                nc.tensor.matmul(mps[0:128, 0:cw], ones[0:1, 0:128],
                                 mrow[0:1, 0:cw], start=True, stop=True,
                                 tile_position=tp)
                mb = sb.tile([128, 512], BF16, tag="mbuf")
                nc.vector.tensor_scalar_mul(mb[:, 0:cw], mps[:, 0:cw], 1.0)
                return mb

            def memset_shell_planes(buf, nkb, locp, YX):
                for kb in range(nkb):
                    for p in (0, locp - 1):
                        off = 0
                        while off < YX:
                            w = min(2048, YX - off)
                            nc.sync.dma_start(
                                buf[kb * 128:(kb + 1) * 128,
                                    p * YX + off:p * YX + off + w],
                                zt[:, 0:w])
                            off += w

            def conv(ci, src, plo, phi, is_final=False):
                l = CONV_LEVEL[ci]
                cin, cout = CONV_CH[ci]
                nkb, nmb = _kb(cin), _kb(cout)
                Zp, Yp, Xp = _dims(l)
                YX = _yx(l)
                slot_n = YX + 2 * (Xp + 1)
                kp = 4 if ci == 0 else 128
                apply_nb = (ci % 2 == 1) and l < LEVELS - 1

                stacked = (ci == 0)
                rowtiled = (cin == 64 and ci != 0)
                if stacked:
                    wt = wpool.tile([128, 128], BF16, tag="wts")
                    nc.sync.dma_start(wt[:, :], w_in[0].ap())
                else:
                    wt = wpool.tile([128, nkb * 27 * nmb * 128], BF16,
                                    tag="wts")
                    nc.sync.dma_start(
                        wt[:, :].rearrange("p (a b) -> p a b", a=nkb),
                        w_in[ci].ap().rearrange("(a p) b -> p a b", p=128))

                offs = [(dz, dy * Xp + dx)
                        for dz in (-1, 0, 1) for dy in (-1, 0, 1)
                        for dx in (-1, 0, 1)]
                nch = (YX + 511) // 512
                dst = cbuf[ci]
                slot_tiles = {}
                for p in range(plo, phi):
                    if stacked:
                        sst = slots_pool.tile([128, YX], BF16,
                                              tag=f"slot{p % 3}")
                        for j, (dz, dfree) in enumerate(offs):
                            st0 = (p + dz) * YX + dfree
                            nc.sync.dma_start(
                                sst[j * 4:j * 4 + 4, 0:YX],
                                src[0:4, st0:st0 + YX])
                        for c in range(nch):
                            c0 = c * 512
                            cw = min(512, YX - c0)
                            mrow = sb.tile([1, 512], BF16, tag="mrow")
                            nc.sync.dma_start(
                                mrow[0:1, 0:cw],
                                mask_in[l].ap()[0:1,
                                                p * YX + c0:p * YX + c0 + cw])
                            mbt = bcast_mask(mrow, cw)
                            acc = ps.tile([128, 512], F32, tag="acc")
                            nc.tensor.matmul(
                                acc[0:128, 0:cw], wt[0:108, 0:128],
                                sst[0:108, c0:c0 + cw],
                                start=True, stop=True)
                            ot = sb.tile([128, 512], BF16, tag="evac")
                            nc.vector.tensor_tensor(
                                ot[:, 0:cw], acc[:, 0:cw], mbt[:, 0:cw],
                                op=mybir.AluOpType.mult)
                            nc.sync.dma_start(
                                dst[0:128, p * YX + c0:p * YX + c0 + cw],
                                ot[:, 0:cw])
                        continue
                    for q in (p - 1, p, p + 1):
                        if q not in slot_tiles:
                            st = slots_pool.tile([128, nkb * slot_n], BF16,
                                                 tag=f"slot{q % 3}")
                            total = _locp(l) * YX
                            for kb in range(nkb):
                                base = q * YX - (Xp + 1)
                                lo = max(base, 0)
                                hi = min(base + slot_n, total)
                                if lo > base or hi < base + slot_n:
                                    off = 0
                                    while off < slot_n:
                                        w = min(2048, slot_n - off)
                                        nc.vector.tensor_copy(
                                            st[0:kp, kb * slot_n + off:
                                               kb * slot_n + off + w],
                                            zt[0:kp, 0:w])
                                        off += w
                                if rowtiled:
                                    nc.sync.dma_start(
                                        st[0:64, (lo - base):(hi - base)],
                                        src[0:64, lo:hi])
                                    nc.sync.dma_start(
                                        st[64:128, (lo - base):(hi - base)],
                                        src[0:64, lo:hi])
                                else:
                                    nc.sync.dma_start(
                                        st[0:kp, kb * slot_n + (lo - base):
                                           kb * slot_n + (hi - base)],
                                        src[kb * 128:kb * 128 + kp, lo:hi])
                            slot_tiles[q] = st
                    for q in list(slot_tiles):
                        if q < p - 1:
                            del slot_tiles[q]
                    if rowtiled:
                        for cb2 in range(0, nch, 2):
                            pair = [c for c in (cb2, cb2 + 1) if c < nch]
                            evs = []
                            for ic, c in enumerate(pair):
                                c0 = c * 512
                                cw = min(512, YX - c0)
                                mrow = sb.tile([1, 512], BF16, tag="mrow")
                                nc.sync.dma_start(
                                    mrow[0:1, 0:cw],
                                    mask_in[l].ap()[0:1, p * YX + c0:
                                                    p * YX + c0 + cw])
                                mbt = bcast_mask(mrow, cw, tp=(0, 0))
                                acc = ps.tile([128, 512], F32,
                                              tag=f"acc{ic}")
                                evs.append((acc, mbt, c0, cw))
                            for k27, (dz, dfree) in enumerate(offs):
                                woff = k27 * 128
                                for ic, c in enumerate(pair):
                                    acc, mbt, c0, cw = evs[ic]
                                    bp = 64 * ic
                                    rb = (Xp + 1) + c0 + dfree
                                    nc.tensor.matmul(
                                        acc[0:128, 0:cw],
                                        wt[bp:bp + 64, woff:woff + 128],
                                        slot_tiles[p + dz][bp:bp + 64,
                                                           rb:rb + cw],
                                        start=(k27 == 0),
                                        stop=(k27 == 26),
                                        tile_position=(bp, 0))
                            for ic, c in enumerate(pair):
                                acc, mbt, c0, cw = evs[ic]
                                ot = sb.tile([128, 512], BF16, tag="evac")
                                nc.vector.tensor_tensor(
                                    ot[:, 0:cw], acc[:, 0:cw], mbt[:, 0:cw],
                                    op=mybir.AluOpType.mult)
                                if apply_nb:
                                    nbr = sb.tile([128, 512], BF16,
                                                  tag="nbuf")
                                    nc.vector.tensor_scalar(
                                        nbr[:, 0:cw], mbt[:, 0:cw], BIG,
                                        -BIG, op0=mybir.AluOpType.mult,
                                        op1=mybir.AluOpType.add)
                                    nc.vector.tensor_tensor(
                                        ot[:, 0:cw], ot[:, 0:cw],
                                        nbr[:, 0:cw],
                                        op=mybir.AluOpType.add)
                                nc.sync.dma_start(
                                    dst[0:128,
                                        p * YX + c0:p * YX + c0 + cw],
                                    ot[:, 0:cw])
                        continue
                    for c in range(nch):
                        c0 = c * 512
                        cw = min(512, YX - c0)
                        mrow = sb.tile([1, 512], BF16, tag="mrow")
                        nc.sync.dma_start(
                            mrow[0:1, 0:cw],
                            mask_in[l].ap()[0:1, p * YX + c0:p * YX + c0 + cw])
                        mbt = bcast_mask(mrow, cw)
                        for mb in range(nmb):
                            acc = ps.tile([128, 512], F32, tag="acc")
                            n_mm = nkb * 27
                            i_mm = 0
                            for kb in range(nkb):
                                for k27, (dz, dfree) in enumerate(offs):
                                    woff = kb * 27 * nmb * 128 \
                                        + (k27 * nmb + mb) * 128
                                    rb = kb * slot_n + (Xp + 1) + c0 + dfree
                                    nc.tensor.matmul(
                                        acc[0:128, 0:cw],
                                        wt[0:kp, woff:woff + 128],
                                        slot_tiles[p + dz][0:kp, rb:rb + cw],
                                        start=(i_mm == 0),
                                        stop=(i_mm == n_mm - 1))
                                    i_mm += 1
                            ot = sb.tile([128, 512],
                                         F32 if is_final else BF16,
                                         tag="evacf" if is_final else "evac")
                            nc.vector.tensor_tensor(
                                ot[:, 0:cw], acc[:, 0:cw], mbt[:, 0:cw],
                                op=mybir.AluOpType.mult)
                            if apply_nb:
                                nbr = sb.tile([128, 512], BF16, tag="nbuf")
                                nc.vector.tensor_scalar(
                                    nbr[:, 0:cw], mbt[:, 0:cw], BIG, -BIG,
                                    op0=mybir.AluOpType.mult,
                                    op1=mybir.AluOpType.add)
                                nc.vector.tensor_tensor(
                                    ot[:, 0:cw], ot[:, 0:cw], nbr[:, 0:cw],
                                    op=mybir.AluOpType.add)
                            tgt = out6.ap() if is_final else dst
                            nc.sync.dma_start(
                                tgt[mb * 128:(mb + 1) * 128,
                                    p * YX + c0:p * YX + c0 + cw],
                                ot[:, 0:cw])

            def pool(l, Plo, Phi, stage=None, write_lin=True):
                """Max-pool level l -> l+1 over next-level local planes
                [Plo, Phi). stage: (stage_tile, plane->slot map)."""
                nl = l + 1
                src_ci = 2 * l + 1
                cout = CONV_CH[src_ci][1]
                nkb = _kb(cout)
                Zp, Yp, Xp = _dims(l)
                YX = _yx(l)
                Yp2, Xp2 = _dims(nl)[1:]
                YX2 = _yx(nl)
                S2 = SIDE[nl]
                src = cbuf[src_ci]
                for P in range(Plo, Phi):
                    pin = (2 * P - 3) if SHARDED[l] else (2 * P - 1)
                    for kb in range(nkb):
                        wa = slots_pool.tile([128, YX], BF16, tag="slot0")
                        wb = slots_pool.tile([128, YX], BF16, tag="slot1")
                        nc.sync.dma_start(
                            wa[:, :], src[kb * 128:(kb + 1) * 128,
                                          pin * YX:(pin + 1) * YX])
                        nc.sync.dma_start(
                            wb[:, :], src[kb * 128:(kb + 1) * 128,
                                          (pin + 1) * YX:(pin + 2) * YX])
                        o = sbp.tile([128, YX2], BF16, tag="pout")
                        nc.vector.memset(o[:, :], 0.0)
                        ov = o[0:128, 0:YX2].rearrange(
                            "p (y x) -> p y x", y=Yp2)[0:128, 1:S2 + 1,
                                                       1:S2 + 1]
                        first = True
                        for oz in (0, 1):
                            wz = (wa, wb)[oz]
                            wv = wz[0:128, 0:YX].rearrange(
                                "p (y x) -> p y x", y=Yp)
                            for oy in (0, 1):
                                for ox in (0, 1):
                                    s_ap = wv[0:128,
                                              1 + oy:1 + oy + 2 * S2:2,
                                              1 + ox:1 + ox + 2 * S2:2]
                                    if first:
                                        nc.vector.tensor_tensor(
                                            ov, s_ap, s_ap,
                                            op=mybir.AluOpType.max)
                                        first = False
                                    else:
                                        nc.vector.tensor_tensor(
                                            ov, ov, s_ap,
                                            op=mybir.AluOpType.max)
                        for c2 in range(0, YX2, 512):
                            w2 = min(512, YX2 - c2)
                            mrow2 = sb.tile([1, 512], BF16, tag="mrow2")
                            nc.sync.dma_start(
                                mrow2[0:1, 0:w2],
                                mask_in[nl].ap()[0:1, P * YX2 + c2:
                                                 P * YX2 + c2 + w2])
                            mb2 = bcast_mask(mrow2, w2)
                            nc.vector.tensor_tensor(
                                o[:, c2:c2 + w2], o[:, c2:c2 + w2],
                                mb2[:, 0:w2],
                                op=mybir.AluOpType.mult)
                        om = o
                        if write_lin:
                            nc.sync.dma_start(
                                lin[nl][kb * 128:(kb + 1) * 128,
                                        P * YX2:(P + 1) * YX2],
                                om[:, :])
                        if stage is not None:
                            st_tile, slotmap = stage
                            if P in slotmap:
                                for s in slotmap[P]:
                                    nc.sync.dma_start(
                                        st_tile[:, (s * nkb + kb) * YX2:
                                                (s * nkb + kb + 1) * YX2],
                                        om[:, :])

            def halo_exchange(nl, stage_tile, nkb):
                """AllGather stage strips and patch halo planes of lin[nl].
                Neighbor selection is static: weighted sum over all 8 ranks
                with per-core 0/1 select weights (avoids dynamic APs)."""
                YX2 = _yx(nl)
                slab2 = _slab(nl)
                W = 4 * nkb * YX2
                ago = dram.tile([8 * 128, W], BF16, tag=f"ago{nl}",
                                name=f"ago{nl}")
                nc.gpsimd.collective_compute(
                    "AllGather", mybir.AluOpType.bypass,
                    replica_groups=[list(range(NCORES))],
                    ins=[stage_tile[:, :].opt()],
                    outs=[ago[:, :].opt()])
                # top halo: rank (c-1)'s bottom strips (slots 2,3) -> planes
                # 1,2; bottom halo: rank (c+1)'s top strips (slots 0,1) ->
                # planes slab2+3, slab2+4
                for side, selw in ((0, sel_top), (1, sel_bot)):
                    for s in (0, 1):
                        slot = (2 + s) if side == 0 else s
                        dstp = (1 + s) if side == 0 else (slab2 + 3 + s)
                        for kb in range(nkb):
                            acc = sbp.tile([128, YX2], BF16, tag="pout")
                            for r in range(NCORES):
                                hb = sbp.tile([128, YX2], BF16, tag="hbx")
                                nc.sync.dma_start(
                                    hb[:, :],
                                    ago[r * 128:(r + 1) * 128,
                                        (slot * nkb + kb) * YX2:
                                        (slot * nkb + kb + 1) * YX2])
                                if r == 0:
                                    nc.vector.tensor_scalar_mul(
                                        acc[:, :], hb[:, :],
                                        selw[0:128, r:r + 1])
                                else:
                                    tmp = sbp.tile([128, YX2], BF16,
                                                   tag="hby")
                                    nc.vector.tensor_scalar_mul(
                                        tmp[:, :], hb[:, :],
                                        selw[0:128, r:r + 1])
                                    nc.vector.tensor_tensor(
                                        acc[:, :], acc[:, :], tmp[:, :],
                                        op=mybir.AluOpType.add)
                            nc.sync.dma_start(
                                lin[nl][kb * 128:(kb + 1) * 128,
                                        dstp * YX2:(dstp + 1) * YX2],
                                acc[:, :])

            # ================= network =================
            # L0
            s0 = _slab(0)
            conv(0, g0.ap(), 2, s0 + 4)
            conv(1, cbuf[0], 3, s0 + 3)
            for l in (0, 1, 2):
                nl = l + 1
                slab2 = _slab(nl)
                nkb2 = _kb(CONV_CH[2 * nl][0])
                stage = dram.tile([128, 4 * nkb2 * _yx(nl)], BF16,
                                  tag=f"stage{nl}")
                sm = {}
                for plane, slot in ((3, 0), (4, 1), (slab2 + 1, 2),
                                    (slab2 + 2, 3)):
                    sm.setdefault(plane, []).append(slot)
                pool(l, 3, slab2 + 3, stage=(stage, sm))
                halo_exchange(nl, stage, nkb2)
                sl2 = _slab(nl)
                conv(2 * nl, lin[nl], 2, sl2 + 4)
                conv(2 * nl + 1, cbuf[2 * nl], 3, sl2 + 3)
            # L3 -> L4 (full gather of single planes)
            nkb4 = _kb(CONV_CH[8][0])
            YX4 = _yx(4)
            stage4 = dram.tile([128, nkb4 * YX4], BF16, tag="stage4")
            sm4 = {3: [0]}
            pool(3, 3, 4, stage=(stage4, sm4), write_lin=False)
            ago4 = dram.tile([8 * 128, nkb4 * YX4], BF16, tag="ago4")
            nc.gpsimd.collective_compute(
                "AllGather", mybir.AluOpType.bypass,
                replica_groups=[list(range(NCORES))],
                ins=[stage4[:, :].opt()],
                outs=[ago4[:, :].opt()])
            memset_shell_planes(lin[4], nkb4, _locp(4), YX4)
            for r in range(NCORES):
                for kb in range(nkb4):
                    hb = sbp.tile([128, YX4], BF16, tag="pout")
                    nc.sync.dma_start(
                        hb[:, :], ago4[r * 128:(r + 1) * 128,
                                       kb * YX4:(kb + 1) * YX4])
                    nc.sync.dma_start(
                        lin[4][kb * 128:(kb + 1) * 128,
                               (r + 1) * YX4:(r + 2) * YX4],
                        hb[:, :])
            # L4-L6 replicated
            for l in (4, 5, 6):
                Zp = _dims(l)[0]
                ca, cb = 2 * l, 2 * l + 1
                memset_shell_planes(cbuf[ca], _kb(CONV_CH[ca][1]), _locp(l),
                                    _yx(l))
                conv(ca, lin[l], 1, Zp - 1)
                conv(cb, cbuf[ca], 1, Zp - 1, is_final=(l == 6))
                if l < 6:
                    Zp2 = _dims(l + 1)[0]
                    memset_shell_planes(lin[l + 1], _kb(CONV_CH[2 * l + 2][0]),
                                        _locp(l + 1), _yx(l + 1))
                    pool(l, 1, Zp2 - 1)

    nc.compile()
    _PROG_CACHE["p"] = nc
    return nc


# ---------------------------------------------------------------------------
# Entry point
# ---------------------------------------------------------------------------

def kernel(**inputs):
    feats = np.asarray(inputs["features"], np.float32)
    coors = np.asarray(inputs["coors"], np.int32)
    ws = [np.asarray(inputs[f"w{i}"], np.float32) for i in range(len(CONV_CH))]

    active, final_sites = build_host_plan(coors)
    wpk = pack_weights(ws)

    # densified, padded level-0 input [4, Zp, Yp, Xp]
    Zp, Yp, Xp = _dims(0)
    dense0 = np.zeros((4, Zp, Yp, Xp), np.float32)
    dense0[0:3, coors[:, 1] + 1, coors[:, 2] + 1, coors[:, 3] + 1] = feats.T

    masks = [_padded_mask(active[l]) for l in range(LEVELS)]
    nbias = [(m - 1.0) * BIG for m in masks]

    nc = build_program()

    in_maps = []
    for core in range(NCORES):
        im = {}
        g0w = np.stack([_core_window(0, core, dense0[c]) for c in range(4)])
        im["grid0"] = g0w.reshape(4, -1).astype(BF16NP)
        for i in range(len(CONV_CH)):
            im[f"w{i}"] = wpk[i]
        for l in range(LEVELS):
            im[f"mask{l}"] = _core_window(l, core, masks[l]) \
                .reshape(1, -1).astype(BF16NP)
            if l < LEVELS - 1:
                im[f"nbias{l}"] = _core_window(l, core, nbias[l]) \
                    .reshape(1, -1).astype(BF16NP)
        st = np.zeros((128, 8), np.float32)
        if core > 0:
            st[:, core - 1] = 1.0
        sb_ = np.zeros((128, 8), np.float32)
        if core < NCORES - 1:
            sb_[:, core + 1] = 1.0
        im["sel_top"] = st
        im["sel_bot"] = sb_
        in_maps.append(im)

    import time as _time
    _t0 = _time.time()
    res = run_bass_kernel_spmd(nc, in_maps, core_ids=list(range(NCORES)))
    global LAST_EXEC_NS, LAST_RUN_WALL_S
    LAST_RUN_WALL_S = _time.time() - _t0
    LAST_EXEC_NS = res.exec_time_ns
    out = res.results[0]["out6"]  # [256, locp6*yx6]

    Yp6, Xp6 = _dims(6)[1:]
    cols = ((final_sites[:, 0] + 1) * Yp6 + final_sites[:, 1] + 1) * Xp6 \
        + final_sites[:, 2] + 1
    return np.ascontiguousarray(out[:, cols].T).astype(np.float32)
